# revision 56
# baseline (speedup 1.0000x reference)
"""Grouped MLP (MoE expert FFN) Bass kernel for 8 Trainium2 NeuronCores.

Problem: 4096 tokens sorted by expert (8 experts, uneven counts), per-expert
GLU MLP:  h = x @ w1[g]  (-> up|gate, 2*2048 cols);  a = silu(up)*gate;
y = a @ w2[g].

Sharding: 2-way token-parallel x 4-way tensor-parallel (INTER split), bf16.
Core (t, q) with t = c//4, q = c%4:
  - token group t owns ~half the tokens (a balanced 4+4 expert partition),
  - inter slice q owns up/gate columns [q*512:(q+1)*512] of every expert
    and the matching w2 rows; its fc2 output is a partial sum of y.
Each core runs an identical program over `slot` token segments whose
lengths are shared between the groups with ZERO padding: the groups'
count vectors are merged by min-of-maxes, letting one expert span two
slots (its weights are simply loaded twice).  Only the DRAM bytes differ
per core.  The host converts everything to bf16, packs weight tiles in
consumption order, and sums the 4 partial y outputs per token group
(host-side reduction; no device collectives).

Device program per core (transposed feature-major space):
  fc1: for slot s, chunk c (<=512 tokens), pair p (128 inter cols):
       up/gate psum accumulated over 8 k-blocks; silu(up)*gate -> a (bf16)
  fc2: one slot behind fc1 (PE never waits on ACT/DVE):
       y^T psum over 4 k-blocks of a; copied bf16 to staging, DMA'd out.
"""

import sys

try:  # concourse normally comes from the container's PYTHONPATH
    import concourse  # noqa: F401
except ImportError:  # pragma: no cover - fallback for stripped env
    for _p in (
        "/root/.axon_site",
        "/root/.axon_site/_ro/trn_rl_repo",
        "/root/.axon_site/_ro/pypackages",
        "/opt/trn_rl_repo",
    ):
        if _p not in sys.path:
            sys.path.append(_p)

from contextlib import ExitStack

import numpy as np
import ml_dtypes

BF16NP = ml_dtypes.bfloat16

NUM_TOKENS = 4096
HIDDEN = 1024
INTER = 2048
GROUPS = 8
N_CORES = 8

NQ = 4              # inter-dim splits
NT = 2              # token-group splits
IW = INTER // NQ    # 512 inter cols per core
NPAIR = IW // 128   # 4 (up,gate) pair blocks
KC = HIDDEN // 128  # 8 k-blocks for fc1
KI = IW // 128      # 4 k-blocks for fc2
MO = HIDDEN // 128  # 8 output row-blocks of y


def _chunks(length):
    # balanced split into <=512-column chunks: N=320+320 pipelines better
    # on the PE than 512+128 (small-N matmuls can't hide their LDWEIGHTS)
    n = max(1, -(-length // 512))
    base, rem = divmod(length, n)
    out, off = [], 0
    for i in range(n):
        c = base + (1 if i < rem else 0)
        out.append((off, c))
        off += c
    return out


_PROGRAM_CACHE: dict = {}


def _build_program(slot_lens):
    """Single-core Bass program; identical NEFF on all 8 cores."""
    import concourse.bass as bass  # noqa: F401
    import concourse.mybir as mybir
    import concourse.tile as tile
    from concourse import bacc

    f32 = mybir.dt.float32
    bf16 = mybir.dt.bfloat16
    silu = mybir.ActivationFunctionType.Silu

    nslot = len(slot_lens)
    tloc = sum(slot_lens)
    soffs = np.concatenate([[0], np.cumsum(slot_lens)]).astype(int)

    nc = bacc.Bacc("TRN2", target_bir_lowering=False, debug=False)

    xt_d = nc.dram_tensor("xt", [128, KC, tloc], bf16, kind="ExternalInput").ap()
    w1_d = nc.dram_tensor(
        "w1b", [nslot, 128, NPAIR * 2 * KC * 128], bf16, kind="ExternalInput"
    ).ap()
    w2_d = nc.dram_tensor(
        "w2b", [nslot, 128, MO * KI * 128], bf16, kind="ExternalInput"
    ).ap()
    yt_d = nc.dram_tensor("yt", [128, MO, tloc], bf16, kind="ExternalOutput").ap()

    with tile.TileContext(nc) as tc, ExitStack() as ctx:
        xp = ctx.enter_context(tc.tile_pool(name="x", bufs=1))
        w1p = ctx.enter_context(tc.tile_pool(name="w1", bufs=2))
        w2p = ctx.enter_context(tc.tile_pool(name="w2", bufs=2))
        ap_ = ctx.enter_context(tc.tile_pool(name="a", bufs=2))
        yp = ctx.enter_context(tc.tile_pool(name="y", bufs=2))
        tp = ctx.enter_context(tc.tile_pool(name="tmp", bufs=4))
        pup = ctx.enter_context(tc.tile_pool(name="pu", bufs=2, space="PSUM"))
        pgp = ctx.enter_context(tc.tile_pool(name="pg", bufs=2, space="PSUM"))
        pyp = ctx.enter_context(tc.tile_pool(name="py", bufs=4, space="PSUM"))

        x_sb = xp.tile([128, KC, tloc], bf16)

        def emit_fc2(s, llen, w2t, a_t, copy_flip, last=False):
            # y-outs ride the scalar engine's HWDGE queue so they drain in
            # parallel with the sync queue's weight/x loads; the final slot
            # streams per-mo so the tail is one small transfer
            y_t = yp.tile([128, MO, llen], bf16)
            for mo in range(MO):
                for coff, clen in _chunks(llen):
                    py = pyp.tile([128, clen], f32)
                    for ki in range(KI):
                        nc.tensor.matmul(
                            py[:, :],
                            w2t[:, (mo * KI + ki) * 128 : (mo * KI + ki + 1) * 128],
                            a_t[:, ki, coff : coff + clen],
                            start=(ki == 0),
                            stop=(ki == KI - 1),
                        )
                    dst = y_t[:, mo, coff : coff + clen]
                    if copy_flip[0]:
                        nc.scalar.copy(dst, py[:, :])
                    else:
                        nc.vector.tensor_copy(dst, py[:, :])
                    copy_flip[0] = not copy_flip[0]
                if last and mo >= MO - 2:
                    # final two rows of the final slot go out per-mo: the
                    # very last transfer is half as big and starts a copy
                    # earlier, trimming the drain after the last matmul
                    nc.sync.dma_start(
                        out=yt_d[:, mo : mo + 1, soffs[s] : soffs[s] + llen],
                        in_=y_t[:, mo : mo + 1, :],
                    )
                elif mo % 2 == 1:  # stream out as mo-pairs complete
                    nc.sync.dma_start(
                        out=yt_d[:, mo - 1 : mo + 1, soffs[s] : soffs[s] + llen],
                        in_=y_t[:, mo - 1 : mo + 1, :],
                    )

        pending = None
        copy_flip = [True]
        emit_order = [s for s in range(nslot) if int(slot_lens[s]) > 0]
        if len(emit_order) >= 2 and slot_lens[emit_order[-1]] < slot_lens[emit_order[-2]]:
            # end on the larger of the two final slots: the very last y
            # writeback then has wider (faster) DMA rows, and the tiny
            # slot's output drains while the final fc2 still computes
            emit_order[-1], emit_order[-2] = emit_order[-2], emit_order[-1]
        for s in emit_order:
            llen = int(slot_lens[s])
            w1t = w1p.tile([128, NPAIR * 2 * KC * 128], bf16)
            hw = KC * 128  # one (pair, half) group of 8 weight tiles

            def w1_dma(lo, hi):
                nc.sync.dma_start(out=w1t[:, lo:hi], in_=w1_d[s, :, lo:hi])

            if s == emit_order[0]:
                # fine-grained trickle so the PE starts as early as possible;
                # pair0's gate weights (g1) slip in mid-x so h=1 doesn't stall
                w1_dma(0, hw)
                for kc in range(KC // 2):
                    nc.sync.dma_start(
                        out=x_sb[:, kc, soffs[s] : soffs[s] + llen],
                        in_=xt_d[:, kc, soffs[s] : soffs[s] + llen],
                    )
                w1_dma(hw, 2 * hw)
                for kc in range(KC // 2, KC):
                    nc.sync.dma_start(
                        out=x_sb[:, kc, soffs[s] : soffs[s] + llen],
                        in_=xt_d[:, kc, soffs[s] : soffs[s] + llen],
                    )
                for g in range(2, 2 * NPAIR):
                    w1_dma(g * hw, (g + 1) * hw)
            else:
                # batched transfers (bigger packets, fewer descriptors)
                nc.sync.dma_start(
                    out=x_sb[:, :, soffs[s] : soffs[s] + llen],
                    in_=xt_d[:, :, soffs[s] : soffs[s] + llen],
                )
                for g in range(0, 2 * NPAIR, 4):
                    w1_dma(g * hw, (g + 4) * hw)
            w2t = w2p.tile([128, MO * KI * 128], bf16)
            nc.sync.dma_start(out=w2t[:, :], in_=w2_d[s])

            a_t = ap_.tile([128, KI, llen], bf16)
            chunks = _chunks(llen)
            for p in range(NPAIR):
                pus = [pup.tile([128, clen], f32, name="pu") for _, clen in chunks]
                pgs = [pgp.tile([128, clen], f32, name="pg") for _, clen in chunks]
                for h, pss in ((0, pus), (1, pgs)):
                    tbase = (p * 2 + h) * KC
                    for kc in range(KC):
                        for ci, (coff, clen) in enumerate(chunks):
                            nc.tensor.matmul(
                                pss[ci][:, :],
                                w1t[:, (tbase + kc) * 128 : (tbase + kc + 1) * 128],
                                x_sb[:, kc, soffs[s] + coff : soffs[s] + coff + clen],
                                start=(kc == 0),
                                stop=(kc == KC - 1),
                            )
                for ci, (coff, clen) in enumerate(chunks):
                    tmp = tp.tile([128, clen], f32)
                    nc.scalar.activation(tmp[:, :], pus[ci][:, :], silu)
                    nc.vector.tensor_mul(
                        a_t[:, p, coff : coff + clen], tmp[:, :], pgs[ci][:, :]
                    )

            if pending is not None:
                emit_fc2(*pending, copy_flip)
            pending = (s, llen, w2t, a_t)
        if pending is not None:
            emit_fc2(*pending, copy_flip, last=True)

    nc.compile()
    return nc


def _get_program(slot_lens):
    key = tuple(int(v) for v in slot_lens)
    if key not in _PROGRAM_CACHE:
        _PROGRAM_CACHE[key] = _build_program(key)
    return _PROGRAM_CACHE[key]


def _partition_experts(counts):
    """Split experts into NT groups (balanced token sums), then merge the
    groups' count vectors into common zero-pad slot lengths.

    Returns (slots, slot_lens): slots[t] is a list of (expert, tok_off, len)
    segments, one per slot, expert == -1 for a pure-padding segment.  Slot
    lengths are produced by repeatedly taking min-of-maxes of the remaining
    per-group segments, so every group fills every slot exactly (an expert
    may span two slots; its weights are then loaded twice)."""
    order = np.argsort(-counts, kind="stable")
    groups = [[] for _ in range(NT)]
    sums = [0] * NT
    cap = GROUPS // NT
    for e in order:
        cand = sorted(range(NT), key=lambda t: (sums[t],))
        for t in cand:
            if len(groups[t]) < cap:
                groups[t].append(int(e))
                sums[t] += int(counts[e])
                break
    # per group: remaining (expert, tok_off, len) pieces; pad the lighter
    # groups with a virtual expert -1 so all groups sum to the max
    target = max(sums)
    rem = []
    for t in range(NT):
        pieces = [(e, 0, int(counts[e])) for e in groups[t] if int(counts[e])]
        if sums[t] < target:
            pieces.append((-1, 0, target - sums[t]))
        rem.append(pieces)

    slots = [[] for _ in range(NT)]
    slot_lens = []
    while any(rem):
        biggest = [max(p, key=lambda x: x[2]) for p in rem]
        ln = min(b[2] for b in biggest)
        slot_lens.append(ln)
        for t in range(NT):
            e, off, l = biggest[t]
            rem[t].remove(biggest[t])
            slots[t].append((e, off, ln))
            if l > ln:
                rem[t].append((e, off + ln, l - ln))
    return slots, tuple(slot_lens)


def _pack_core_inputs(x, w1, w2, offs, slots, slot_lens):
    """Per-core DRAM blobs (bf16), shared xt per token group."""
    nslot = len(slot_lens)
    tloc = int(sum(slot_lens))
    soffs = np.concatenate([[0], np.cumsum(slot_lens)]).astype(int)

    xts = []
    for t in range(NT):
        xt = np.zeros((128, KC, tloc), BF16NP)
        for i, (e, toff, ln) in enumerate(slots[t]):
            if e < 0:
                continue
            seg = x[offs[e] + toff : offs[e] + toff + ln].T.astype(BF16NP)
            xt[:, :, soffs[i] : soffs[i] + ln] = seg.reshape(KC, 128, ln).transpose(
                1, 0, 2
            )
        xts.append(xt)

    in_maps = []
    for c in range(N_CORES):
        t, q = divmod(c, NQ)
        w1b = np.empty((nslot, 128, NPAIR * 2 * KC * 128), BF16NP)
        w2b = np.empty((nslot, 128, MO * KI * 128), BF16NP)
        for i, (e, toff, ln) in enumerate(slots[t]):
            if e < 0:
                w1b[i] = 0
                w2b[i] = 0
                continue
            up = w1[e][:, q * IW : (q + 1) * IW]
            gate = w1[e][:, INTER + q * IW : INTER + (q + 1) * IW]
            hs = np.stack([up, gate], 0).astype(BF16NP)  # [2, 1024, 512]
            hs = hs.reshape(2, KC, 128, NPAIR, 128).transpose(2, 3, 0, 1, 4)
            w1b[i] = hs.reshape(128, NPAIR * 2 * KC * 128)
            sl = w2[e][q * IW : (q + 1) * IW, :].astype(BF16NP)  # [512, 1024]
            sl = sl.reshape(KI, 128, MO, 128).transpose(1, 2, 0, 3)
            w2b[i] = sl.reshape(128, MO * KI * 128)
        in_maps.append({"xt": xts[t], "w1b": w1b, "w2b": w2b})
    return in_maps


_LAST_RESULTS = {}  # exposed for test.py (exec time, trace paths)


def kernel(permuted_tokens, tokens_per_expert, w1, w2, _trace=False):
    from concourse.bass_utils import run_bass_kernel_spmd

    x = np.asarray(permuted_tokens, np.float32)
    counts = np.asarray(tokens_per_expert, np.int64)
    w1 = np.asarray(w1, np.float32)
    w2 = np.asarray(w2, np.float32)

    offs = np.zeros(GROUPS + 1, np.int64)
    offs[1:] = np.cumsum(counts)

    slots, slot_lens = _partition_experts(counts)
    nc = _get_program(slot_lens)
    in_maps = _pack_core_inputs(x, w1, w2, offs, slots, slot_lens)

    kwargs = {}
    if _trace:
        kwargs = dict(trace=True, trace_cores=list(range(N_CORES)))
    res = run_bass_kernel_spmd(nc, in_maps, core_ids=list(range(N_CORES)), **kwargs)
    _LAST_RESULTS["res"] = res

    soffs = np.concatenate([[0], np.cumsum(slot_lens)]).astype(int)
    out = np.empty((NUM_TOKENS, HIDDEN), np.float32)
    for t in range(NT):
        acc = np.zeros((128, MO, int(sum(slot_lens))), np.float32)
        for q in range(NQ):
            acc += res.results[t * NQ + q]["yt"].astype(np.float32)
        ymat = acc.transpose(1, 0, 2).reshape(HIDDEN, -1)  # [1024, tloc]
        for i, (e, toff, ln) in enumerate(slots[t]):
            if e < 0:
                continue
            out[offs[e] + toff : offs[e] + toff + ln] = (
                ymat[:, soffs[i] : soffs[i] + ln].T
            )
    return out


# revision 57
# speedup vs baseline: 1.0250x; 1.0250x over previous
"""Grouped MLP (MoE expert FFN) Bass kernel for 8 Trainium2 NeuronCores.

Problem: 4096 tokens sorted by expert (8 experts, uneven counts), per-expert
GLU MLP:  h = x @ w1[g]  (-> up|gate, 2*2048 cols);  a = silu(up)*gate;
y = a @ w2[g].

Sharding: 2-way token-parallel x 4-way tensor-parallel (INTER split), bf16.
Core (t, q) with t = c//4, q = c%4:
  - token group t owns ~half the tokens (a balanced 4+4 expert partition),
  - inter slice q owns up/gate columns [q*512:(q+1)*512] of every expert
    and the matching w2 rows; its fc2 output is a partial sum of y.
Each core runs an identical program over `slot` token segments whose
lengths are shared between the groups with ZERO padding: the groups'
count vectors are merged by min-of-maxes, letting one expert span two
slots (its weights are simply loaded twice).  Only the DRAM bytes differ
per core.  The host converts everything to bf16, packs weight tiles in
consumption order, and sums the 4 partial y outputs per token group
(host-side reduction; no device collectives).

Device program per core (transposed feature-major space):
  fc1: for slot s, chunk c (<=512 tokens), pair p (128 inter cols):
       up/gate psum accumulated over 8 k-blocks; silu(up)*gate -> a (bf16)
  fc2: one slot behind fc1 (PE never waits on ACT/DVE):
       y^T psum over 4 k-blocks of a; copied bf16 to staging, DMA'd out.
"""

import sys

try:  # concourse normally comes from the container's PYTHONPATH
    import concourse  # noqa: F401
except ImportError:  # pragma: no cover - fallback for stripped env
    for _p in (
        "/root/.axon_site",
        "/root/.axon_site/_ro/trn_rl_repo",
        "/root/.axon_site/_ro/pypackages",
        "/opt/trn_rl_repo",
    ):
        if _p not in sys.path:
            sys.path.append(_p)

from contextlib import ExitStack

import numpy as np
import ml_dtypes

BF16NP = ml_dtypes.bfloat16

NUM_TOKENS = 4096
HIDDEN = 1024
INTER = 2048
GROUPS = 8
N_CORES = 8

NQ = 4              # inter-dim splits
NT = 2              # token-group splits
IW = INTER // NQ    # 512 inter cols per core
NPAIR = IW // 128   # 4 (up,gate) pair blocks
KC = HIDDEN // 128  # 8 k-blocks for fc1
KI = IW // 128      # 4 k-blocks for fc2
MO = HIDDEN // 128  # 8 output row-blocks of y


def _chunks(length):
    # balanced split into <=512-column chunks: N=320+320 pipelines better
    # on the PE than 512+128 (small-N matmuls can't hide their LDWEIGHTS)
    n = max(1, -(-length // 512))
    base, rem = divmod(length, n)
    out, off = [], 0
    for i in range(n):
        c = base + (1 if i < rem else 0)
        out.append((off, c))
        off += c
    return out


_PROGRAM_CACHE: dict = {}


def _build_program(slot_lens):
    """Single-core Bass program; identical NEFF on all 8 cores."""
    import concourse.bass as bass  # noqa: F401
    import concourse.mybir as mybir
    import concourse.tile as tile
    from concourse import bacc

    f32 = mybir.dt.float32
    bf16 = mybir.dt.bfloat16
    silu = mybir.ActivationFunctionType.Silu

    nslot = len(slot_lens)
    tloc = sum(slot_lens)
    soffs = np.concatenate([[0], np.cumsum(slot_lens)]).astype(int)

    nc = bacc.Bacc("TRN2", target_bir_lowering=False, debug=False)

    xt_d = nc.dram_tensor("xt", [128, KC, tloc], bf16, kind="ExternalInput").ap()
    w1_d = nc.dram_tensor(
        "w1b", [nslot, 128, NPAIR * 2 * KC * 128], bf16, kind="ExternalInput"
    ).ap()
    w2_d = nc.dram_tensor(
        "w2b", [nslot, 128, MO * KI * 128], bf16, kind="ExternalInput"
    ).ap()
    yt_d = nc.dram_tensor("yt", [128, MO, tloc], bf16, kind="ExternalOutput").ap()

    with tile.TileContext(nc) as tc, ExitStack() as ctx:
        xp = ctx.enter_context(tc.tile_pool(name="x", bufs=1))
        w1p = ctx.enter_context(tc.tile_pool(name="w1", bufs=2))
        w2p = ctx.enter_context(tc.tile_pool(name="w2", bufs=2))
        ap_ = ctx.enter_context(tc.tile_pool(name="a", bufs=2))
        yp = ctx.enter_context(tc.tile_pool(name="y", bufs=2))
        tp = ctx.enter_context(tc.tile_pool(name="tmp", bufs=4))
        pup = ctx.enter_context(tc.tile_pool(name="pu", bufs=2, space="PSUM"))
        pgp = ctx.enter_context(tc.tile_pool(name="pg", bufs=2, space="PSUM"))
        pyp = ctx.enter_context(tc.tile_pool(name="py", bufs=4, space="PSUM"))

        x_sb = xp.tile([128, KC, tloc], bf16)

        def emit_fc2(s, llen, w2t, a_t, copy_flip, last=False):
            # y-outs ride the scalar engine's HWDGE queue so they drain in
            # parallel with the sync queue's weight/x loads; the final slot
            # streams per-mo so the tail is one small transfer
            y_t = yp.tile([128, MO, llen], bf16)
            for mo in range(MO):
                for coff, clen in _chunks(llen):
                    py = pyp.tile([128, clen], f32)
                    for ki in range(KI):
                        nc.tensor.matmul(
                            py[:, :],
                            w2t[:, (mo * KI + ki) * 128 : (mo * KI + ki + 1) * 128],
                            a_t[:, ki, coff : coff + clen],
                            start=(ki == 0),
                            stop=(ki == KI - 1),
                        )
                    dst = y_t[:, mo, coff : coff + clen]
                    if copy_flip[0]:
                        nc.scalar.copy(dst, py[:, :])
                    else:
                        nc.vector.tensor_copy(dst, py[:, :])
                    copy_flip[0] = not copy_flip[0]
                if mo % 2 == 1:  # stream out as mo-pairs complete
                    nc.sync.dma_start(
                        out=yt_d[:, mo - 1 : mo + 1, soffs[s] : soffs[s] + llen],
                        in_=y_t[:, mo - 1 : mo + 1, :],
                    )

        pending = None
        copy_flip = [True]
        emit_order = [s for s in range(nslot) if int(slot_lens[s]) > 0]
        if len(emit_order) >= 2 and slot_lens[emit_order[-1]] < slot_lens[emit_order[-2]]:
            # end on the larger of the two final slots: the very last y
            # writeback then has wider (faster) DMA rows, and the tiny
            # slot's output drains while the final fc2 still computes
            emit_order[-1], emit_order[-2] = emit_order[-2], emit_order[-1]
        for s in emit_order:
            llen = int(slot_lens[s])
            w1t = w1p.tile([128, NPAIR * 2 * KC * 128], bf16)
            hw = KC * 128  # one (pair, half) group of 8 weight tiles

            def w1_dma(lo, hi):
                nc.sync.dma_start(out=w1t[:, lo:hi], in_=w1_d[s, :, lo:hi])

            if s == emit_order[0]:
                # fine-grained trickle so the PE starts as early as possible;
                # pair0's gate weights (g1) slip in mid-x so h=1 doesn't stall
                w1_dma(0, hw)
                for kc in range(KC // 2):
                    nc.sync.dma_start(
                        out=x_sb[:, kc, soffs[s] : soffs[s] + llen],
                        in_=xt_d[:, kc, soffs[s] : soffs[s] + llen],
                    )
                w1_dma(hw, 2 * hw)
                for kc in range(KC // 2, KC):
                    nc.sync.dma_start(
                        out=x_sb[:, kc, soffs[s] : soffs[s] + llen],
                        in_=xt_d[:, kc, soffs[s] : soffs[s] + llen],
                    )
                for g in range(2, 2 * NPAIR):
                    w1_dma(g * hw, (g + 1) * hw)
            else:
                # batched transfers (bigger packets, fewer descriptors)
                nc.sync.dma_start(
                    out=x_sb[:, :, soffs[s] : soffs[s] + llen],
                    in_=xt_d[:, :, soffs[s] : soffs[s] + llen],
                )
                for g in range(0, 2 * NPAIR, 4):
                    w1_dma(g * hw, (g + 4) * hw)
            w2t = w2p.tile([128, MO * KI * 128], bf16)
            nc.sync.dma_start(out=w2t[:, :], in_=w2_d[s])

            a_t = ap_.tile([128, KI, llen], bf16)
            chunks = _chunks(llen)
            for p in range(NPAIR):
                pus = [pup.tile([128, clen], f32, name="pu") for _, clen in chunks]
                pgs = [pgp.tile([128, clen], f32, name="pg") for _, clen in chunks]
                for h, pss in ((0, pus), (1, pgs)):
                    tbase = (p * 2 + h) * KC
                    for kc in range(KC):
                        for ci, (coff, clen) in enumerate(chunks):
                            nc.tensor.matmul(
                                pss[ci][:, :],
                                w1t[:, (tbase + kc) * 128 : (tbase + kc + 1) * 128],
                                x_sb[:, kc, soffs[s] + coff : soffs[s] + coff + clen],
                                start=(kc == 0),
                                stop=(kc == KC - 1),
                            )
                for ci, (coff, clen) in enumerate(chunks):
                    tmp = tp.tile([128, clen], f32)
                    nc.scalar.activation(tmp[:, :], pus[ci][:, :], silu)
                    nc.vector.tensor_mul(
                        a_t[:, p, coff : coff + clen], tmp[:, :], pgs[ci][:, :]
                    )

            if pending is not None:
                emit_fc2(*pending, copy_flip)
            pending = (s, llen, w2t, a_t)
        if pending is not None:
            emit_fc2(*pending, copy_flip, last=True)

    nc.compile()
    return nc


def _get_program(slot_lens):
    key = tuple(int(v) for v in slot_lens)
    if key not in _PROGRAM_CACHE:
        _PROGRAM_CACHE[key] = _build_program(key)
    return _PROGRAM_CACHE[key]


def _partition_experts(counts):
    """Split experts into NT groups (balanced token sums), then merge the
    groups' count vectors into common zero-pad slot lengths.

    Returns (slots, slot_lens): slots[t] is a list of (expert, tok_off, len)
    segments, one per slot, expert == -1 for a pure-padding segment.  Slot
    lengths are produced by repeatedly taking min-of-maxes of the remaining
    per-group segments, so every group fills every slot exactly (an expert
    may span two slots; its weights are then loaded twice)."""
    order = np.argsort(-counts, kind="stable")
    groups = [[] for _ in range(NT)]
    sums = [0] * NT
    cap = GROUPS // NT
    for e in order:
        cand = sorted(range(NT), key=lambda t: (sums[t],))
        for t in cand:
            if len(groups[t]) < cap:
                groups[t].append(int(e))
                sums[t] += int(counts[e])
                break
    # per group: remaining (expert, tok_off, len) pieces; pad the lighter
    # groups with a virtual expert -1 so all groups sum to the max
    target = max(sums)
    rem = []
    for t in range(NT):
        pieces = [(e, 0, int(counts[e])) for e in groups[t] if int(counts[e])]
        if sums[t] < target:
            pieces.append((-1, 0, target - sums[t]))
        rem.append(pieces)

    slots = [[] for _ in range(NT)]
    slot_lens = []
    while any(rem):
        biggest = [max(p, key=lambda x: x[2]) for p in rem]
        ln = min(b[2] for b in biggest)
        slot_lens.append(ln)
        for t in range(NT):
            e, off, l = biggest[t]
            rem[t].remove(biggest[t])
            slots[t].append((e, off, ln))
            if l > ln:
                rem[t].append((e, off + ln, l - ln))
    return slots, tuple(slot_lens)


def _pack_core_inputs(x, w1, w2, offs, slots, slot_lens):
    """Per-core DRAM blobs (bf16), shared xt per token group."""
    nslot = len(slot_lens)
    tloc = int(sum(slot_lens))
    soffs = np.concatenate([[0], np.cumsum(slot_lens)]).astype(int)

    xts = []
    for t in range(NT):
        xt = np.zeros((128, KC, tloc), BF16NP)
        for i, (e, toff, ln) in enumerate(slots[t]):
            if e < 0:
                continue
            seg = x[offs[e] + toff : offs[e] + toff + ln].T.astype(BF16NP)
            xt[:, :, soffs[i] : soffs[i] + ln] = seg.reshape(KC, 128, ln).transpose(
                1, 0, 2
            )
        xts.append(xt)

    in_maps = []
    for c in range(N_CORES):
        t, q = divmod(c, NQ)
        w1b = np.empty((nslot, 128, NPAIR * 2 * KC * 128), BF16NP)
        w2b = np.empty((nslot, 128, MO * KI * 128), BF16NP)
        for i, (e, toff, ln) in enumerate(slots[t]):
            if e < 0:
                w1b[i] = 0
                w2b[i] = 0
                continue
            up = w1[e][:, q * IW : (q + 1) * IW]
            gate = w1[e][:, INTER + q * IW : INTER + (q + 1) * IW]
            hs = np.stack([up, gate], 0).astype(BF16NP)  # [2, 1024, 512]
            hs = hs.reshape(2, KC, 128, NPAIR, 128).transpose(2, 3, 0, 1, 4)
            w1b[i] = hs.reshape(128, NPAIR * 2 * KC * 128)
            sl = w2[e][q * IW : (q + 1) * IW, :].astype(BF16NP)  # [512, 1024]
            sl = sl.reshape(KI, 128, MO, 128).transpose(1, 2, 0, 3)
            w2b[i] = sl.reshape(128, MO * KI * 128)
        in_maps.append({"xt": xts[t], "w1b": w1b, "w2b": w2b})
    return in_maps


_LAST_RESULTS = {}  # exposed for test.py (exec time, trace paths)


def kernel(permuted_tokens, tokens_per_expert, w1, w2, _trace=False):
    from concourse.bass_utils import run_bass_kernel_spmd

    x = np.asarray(permuted_tokens, np.float32)
    counts = np.asarray(tokens_per_expert, np.int64)
    w1 = np.asarray(w1, np.float32)
    w2 = np.asarray(w2, np.float32)

    offs = np.zeros(GROUPS + 1, np.int64)
    offs[1:] = np.cumsum(counts)

    slots, slot_lens = _partition_experts(counts)
    nc = _get_program(slot_lens)
    in_maps = _pack_core_inputs(x, w1, w2, offs, slots, slot_lens)

    kwargs = {}
    if _trace:
        kwargs = dict(trace=True, trace_cores=list(range(N_CORES)))
    res = run_bass_kernel_spmd(nc, in_maps, core_ids=list(range(N_CORES)), **kwargs)
    _LAST_RESULTS["res"] = res

    soffs = np.concatenate([[0], np.cumsum(slot_lens)]).astype(int)
    out = np.empty((NUM_TOKENS, HIDDEN), np.float32)
    for t in range(NT):
        acc = np.zeros((128, MO, int(sum(slot_lens))), np.float32)
        for q in range(NQ):
            acc += res.results[t * NQ + q]["yt"].astype(np.float32)
        ymat = acc.transpose(1, 0, 2).reshape(HIDDEN, -1)  # [1024, tloc]
        for i, (e, toff, ln) in enumerate(slots[t]):
            if e < 0:
                continue
            out[offs[e] + toff : offs[e] + toff + ln] = (
                ymat[:, soffs[i] : soffs[i] + ln].T
            )
    return out
